# revision 1
# baseline (speedup 1.0000x reference)
"""GCNII (8 layers, N=50000, E=800000) on 8 trn2 NeuronCores.

Sharding: nodes partitioned into 8 contiguous ranges (6250/core); edges
partitioned by destination so each core owns the scatter-add for its node
range. Per layer: AllGather h -> HBM h_full; each core dma_gathers the
source rows for its edges, scatters them into PSUM via one-hot matmuls
(128-dst windows, norm folded into the one-hot), then applies the dense
epilogue with the layer matrix folded as M = (1-beta)I + beta*W on host.
"""
import numpy as np
import concourse.bass as bass
import concourse.mybir as mybir
from concourse import bacc, tile
from concourse.bass_utils import run_bass_kernel_spmd

mdt = mybir.dt

N = 50000
E = 800000
FIN = 128
HID = 64
L = 8
ALPHA = 0.1
THETA = 0.5
NCORES = 8
NS = N // NCORES            # 6250 nodes per core
NW = (NS + 127) // 128      # 49 windows per core
NSPAD = NW * 128            # 6272
CW = 7                      # windows per chunk
CHUNKS = NW // CW           # 7 chunks
assert CW * CHUNKS == NW
HALF = 32768                # int16 gather index split


def _preprocess(x, edge_index, w_in, b_in, conv_w, w_out, b_out):
    row = np.asarray(edge_index[0], dtype=np.int64)
    col = np.asarray(edge_index[1], dtype=np.int64)
    loops = np.arange(N, dtype=np.int64)
    row = np.concatenate([row, loops])
    col = np.concatenate([col, loops])
    deg = np.bincount(col, minlength=N).astype(np.float32)
    dinv = (1.0 / np.sqrt(deg)).astype(np.float32)
    norm = (dinv[row] * dinv[col]).astype(np.float32)

    per_core = []
    for c in range(NCORES):
        m = (col >= c * NS) & (col < (c + 1) * NS)
        r = row[m]
        d = col[m] - c * NS
        nv = norm[m]
        o = np.argsort(d, kind="stable")
        per_core.append((r[o], d[o], nv[o]))

    # per-window lo/hi counts; tile counts shared across cores
    counts = np.zeros((NCORES, NW, 2), dtype=np.int64)
    bounds = []
    for c in range(NCORES):
        r, d, nv = per_core[c]
        wb = np.searchsorted(d, np.arange(0, NSPAD + 1, 128))
        bounds.append(wb)
        for w in range(NW):
            seg = slice(wb[w], wb[w + 1])
            nlo = int((r[seg] < HALF).sum())
            counts[c, w] = (nlo, (wb[w + 1] - wb[w]) - nlo)
    TLs = np.maximum(np.ceil(counts[:, :, 0].max(axis=0) / 128), 1).astype(np.int64)
    THs = np.maximum(np.ceil(counts[:, :, 1].max(axis=0) / 128), 1).astype(np.int64)

    # global tile order: chunks; within chunk all lo tiles (window-major)
    # then all hi tiles (window-major)
    gidx_lo = np.zeros(NW, dtype=np.int64)
    gidx_hi = np.zeros(NW, dtype=np.int64)
    g = 0
    for s in range(CHUNKS):
        for wi in range(CW):
            w = s * CW + wi
            gidx_lo[w] = g
            g += TLs[w]
        for wi in range(CW):
            w = s * CW + wi
            gidx_hi[w] = g
            g += THs[w]
    T = g

    dstw = np.full((NCORES, 128, T), -1.0, dtype=np.float32)
    nrm = np.zeros((NCORES, 128, T), dtype=np.float32)
    idx16 = np.zeros((NCORES, 128, 8 * T), dtype=np.int16)
    for c in range(NCORES):
        r, d, nv = per_core[c]
        wb = bounds[c]
        for w in range(NW):
            seg = slice(wb[w], wb[w + 1])
            rs, ds, ns_ = r[seg], d[seg], nv[seg]
            mlo = rs < HALF
            for p in range(2):
                mask = mlo if p == 0 else ~mlo
                rr = rs[mask] - (0 if p == 0 else HALF)
                dd = ds[mask] - w * 128
                nn = ns_[mask]
                TT = int(TLs[w] if p == 0 else THs[w])
                g0 = int(gidx_lo[w] if p == 0 else gidx_hi[w])
                cap = TT * 128
                rrp = np.zeros(cap, np.int64)
                rrp[: len(rr)] = rr
                ddp = np.full(cap, -1.0, np.float32)
                ddp[: len(dd)] = dd
                nnp = np.zeros(cap, np.float32)
                nnp[: len(nn)] = nn
                for t in range(TT):
                    gg = g0 + t
                    dstw[c, :, gg] = ddp[t * 128 : (t + 1) * 128]
                    nrm[c, :, gg] = nnp[t * 128 : (t + 1) * 128]
                    v = rrp[t * 128 : (t + 1) * 128].astype(np.int16)
                    idx16[c, :, 8 * gg : 8 * (gg + 1)] = np.tile(
                        v.reshape(8, 16).T, (8, 1)
                    )

    # dense weights (shared across cores)
    w_in = np.asarray(w_in, np.float32)
    conv_w = np.asarray(conv_w, np.float32)
    w_out = np.asarray(w_out, np.float32)
    b_in = np.asarray(b_in, np.float32)
    b_out = np.asarray(b_out, np.float32)
    betas = np.log(THETA / np.arange(1, L + 1, dtype=np.float32) + 1.0)
    convT = np.concatenate(
        [
            ((1.0 - betas[l]) * np.eye(HID, dtype=np.float32) + betas[l] * conv_w[l]).T
            for l in range(L)
        ],
        axis=1,
    )  # [64, L*64]
    consts = {
        "w_inT": np.ascontiguousarray(w_in.T),              # [128, 64]
        "convT": np.ascontiguousarray(convT),               # [64, 512]
        "w_outT": np.ascontiguousarray(w_out.T),            # [64, 64]
        "b_in_rep": np.tile(b_in[None, :], (128, 1)),       # [128, 64]
        "b_out_rep": np.tile(b_out[None, :], (128, 1)),     # [128, 64]
        "iota": np.tile(np.arange(128, dtype=np.float32), (128, 1)),
        "ident": np.eye(128, dtype=np.float32),
    }

    x = np.asarray(x, np.float32)
    in_maps = []
    for c in range(NCORES):
        xT = np.zeros((FIN, NSPAD), np.float32)
        xT[:, :NS] = x[c * NS : (c + 1) * NS].T
        in_maps.append(
            dict(
                consts,
                xT=np.ascontiguousarray(xT),
                idx16=np.ascontiguousarray(idx16[c]),
                dstw=np.ascontiguousarray(dstw[c]),
                nrm=np.ascontiguousarray(nrm[c]),
            )
        )
    return in_maps, TLs, THs, gidx_lo, gidx_hi, T


def _build(TLs, THs, gidx_lo, gidx_hi, T, reps=1, sim_single=False):
    nc = bacc.Bacc(None, target_bir_lowering=False, num_devices=NCORES, num_swdge_queues=4)

    xT_in = nc.dram_tensor("xT", [FIN, NSPAD], mdt.float32, kind="ExternalInput")
    idx_in = nc.dram_tensor("idx16", [128, 8 * T], mdt.int16, kind="ExternalInput")
    dstw_in = nc.dram_tensor("dstw", [128, T], mdt.float32, kind="ExternalInput")
    nrm_in = nc.dram_tensor("nrm", [128, T], mdt.float32, kind="ExternalInput")
    w_inT_in = nc.dram_tensor("w_inT", [FIN, HID], mdt.float32, kind="ExternalInput")
    convT_in = nc.dram_tensor("convT", [HID, L * HID], mdt.float32, kind="ExternalInput")
    w_outT_in = nc.dram_tensor("w_outT", [HID, HID], mdt.float32, kind="ExternalInput")
    b_in_in = nc.dram_tensor("b_in_rep", [128, HID], mdt.float32, kind="ExternalInput")
    b_out_in = nc.dram_tensor("b_out_rep", [128, HID], mdt.float32, kind="ExternalInput")
    iota_in = nc.dram_tensor("iota", [128, 128], mdt.float32, kind="ExternalInput")
    ident_in = nc.dram_tensor("ident", [128, 128], mdt.float32, kind="ExternalInput")

    out_t = nc.dram_tensor("out", [NS, HID], mdt.float32, kind="ExternalOutput")

    bounce = nc.dram_tensor("bounce", [NS, HID], mdt.float32)
    h_full = nc.dram_tensor("h_full", [N, HID], mdt.float32, addr_space="Shared")

    # per-chunk sizes and offsets
    NLO = [int(TLs[s * CW : (s + 1) * CW].sum()) for s in range(CHUNKS)]
    NHI = [int(THs[s * CW : (s + 1) * CW].sum()) for s in range(CHUNKS)]
    MAXLO, MAXHI = max(NLO), max(NHI)

    with tile.TileContext(nc) as tc, \
         tc.tile_pool(name="const", bufs=1) as cpool, \
         tc.tile_pool(name="gath", bufs=2) as gpool, \
         tc.tile_pool(name="oh", bufs=4) as ohpool, \
         tc.tile_pool(name="work", bufs=3) as wpool, \
         tc.tile_pool(name="ps_sc", bufs=2, space="PSUM") as psum_sc, \
         tc.tile_pool(name="ps_tr", bufs=2, space="PSUM") as psum_tr, \
         tc.tile_pool(name="ps_mm", bufs=2, space="PSUM") as psum_mm:

        # ---- persistent constants ----
        iota_t = cpool.tile([128, 128], mdt.float32)
        nc.sync.dma_start(iota_t[:], iota_in[:])
        ident_t = cpool.tile([128, 128], mdt.float32)
        nc.sync.dma_start(ident_t[:], ident_in[:])
        w_inT_t = cpool.tile([FIN, HID], mdt.float32)
        nc.sync.dma_start(w_inT_t[:], w_inT_in[:])
        convT_t = cpool.tile([HID, L * HID], mdt.float32)
        nc.sync.dma_start(convT_t[:], convT_in[:])
        w_outT_t = cpool.tile([HID, HID], mdt.float32)
        nc.sync.dma_start(w_outT_t[:], w_outT_in[:])
        b_in_t = cpool.tile([128, HID], mdt.float32)
        nc.sync.dma_start(b_in_t[:], b_in_in[:])
        b_out_t = cpool.tile([128, HID], mdt.float32)
        nc.sync.dma_start(b_out_t[:], b_out_in[:])
        idx_t = cpool.tile([128, 8 * T], mdt.int16)
        nc.sync.dma_start(idx_t[:], idx_in[:])
        dstw_t = cpool.tile([128, T], mdt.float32)
        nc.sync.dma_start(dstw_t[:], dstw_in[:])
        nrm_t = cpool.tile([128, T], mdt.float32)
        nc.sync.dma_start(nrm_t[:], nrm_in[:])
        xT_t = cpool.tile([FIN, NSPAD], mdt.float32)
        nc.sync.dma_start(xT_t[:], xT_in[:])

        h_sb = cpool.tile([128, NW * HID], mdt.float32)
        x0s = cpool.tile([128, NW * HID], mdt.float32)

        def store_h(w):
            nrows = min(NS - w * 128, 128)
            nc.sync.dma_start(
                bounce[w * 128 : w * 128 + nrows, :],
                h_sb[:nrows, w * HID : (w + 1) * HID],
            )

        for rep_i in range(reps):
            # ---- h0 = relu(x @ w_in.T + b_in); x0s = ALPHA * h0 ----
            for w in range(NW):
                ps = psum_mm.tile([128, HID], mdt.float32, tag="mm")
                nc.tensor.matmul(
                    ps[:], xT_t[:, w * 128 : (w + 1) * 128], w_inT_t[:],
                    start=True, stop=True,
                )
                hw = h_sb[:, w * HID : (w + 1) * HID]
                u = wpool.tile([128, HID], mdt.float32, tag="u")
                nc.vector.tensor_tensor(u[:], ps[:], b_in_t[:], mybir.AluOpType.add)
                nc.scalar.activation(hw, u[:], mybir.ActivationFunctionType.Relu)
                nc.vector.tensor_scalar_mul(x0s[:, w * HID : (w + 1) * HID], hw, ALPHA)
                store_h(w)

            def allgather():
                if sim_single:
                    nc.sync.dma_start(h_full[:NS, :], bounce[:])
                else:
                    nc.gpsimd.collective_compute(
                        "AllGather", mybir.AluOpType.bypass,
                        replica_groups=[list(range(NCORES))],
                        ins=[bounce[:]], outs=[h_full[:]],
                    )

            allgather()

            # ---- layers ----
            qctr = [0]
            for l in range(L):
                for s in range(CHUNKS):
                    nlo, nhi = NLO[s], NHI[s]
                    base = int(gidx_lo[s * CW])  # first tile of this chunk
                    glo = gpool.tile([128, MAXLO, HID], mdt.float32, tag="glo")
                    ghi = gpool.tile([128, MAXHI, HID], mdt.float32, tag="ghi")

                    def one_gather(dst, src_ap, tile0, ntiles, nsplit=8):
                        bnds = [tile0 + (ntiles * i) // nsplit for i in range(nsplit + 1)]
                        for i in range(nsplit):
                            a, b = bnds[i], bnds[i + 1]
                            if a == b:
                                continue
                            nidx = (b - a) * 128
                            nc.gpsimd.dma_gather(
                                dst[:, a - tile0 : b - tile0, :], src_ap,
                                idx_t[:, 8 * a : 8 * b], nidx, nidx, HID,
                                single_packet=False, queue_num=qctr[0] % 4,
                            )
                            qctr[0] += 1

                    one_gather(glo, h_full[:, :], base, nlo)
                    one_gather(ghi, h_full[HALF:, :], base + nlo, nhi)
                    for wi in range(CW):
                        w = s * CW + wi
                        ntiles = int(TLs[w] + THs[w])
                        ps = psum_sc.tile([128, HID], mdt.float32, tag="sc")
                        k = 0
                        for p in range(2):
                            TT = int(TLs[w] if p == 0 else THs[w])
                            g0 = int(gidx_lo[w] if p == 0 else gidx_hi[w])
                            gsrc = glo if p == 0 else ghi
                            for t in range(TT):
                                gg = g0 + t
                                slot = gg - base if p == 0 else gg - base - nlo
                                oh = ohpool.tile([128, 128], mdt.float32, tag="oh")
                                nc.vector.tensor_scalar(
                                    oh[:], iota_t[:],
                                    dstw_t[:, gg : gg + 1], nrm_t[:, gg : gg + 1],
                                    mybir.AluOpType.is_equal, mybir.AluOpType.mult,
                                )
                                nc.tensor.matmul(
                                    ps[:], oh[:], gsrc[:, slot, :],
                                    start=(k == 0), stop=(k == ntiles - 1),
                                )
                                k += 1
                        # z = 0.9 * ps + x0s ; h = relu(z @ M_l.T)
                        zw = wpool.tile([128, HID], mdt.float32, tag="zw")
                        nc.vector.scalar_tensor_tensor(
                            zw[:], ps[:], 1.0 - ALPHA, x0s[:, w * HID : (w + 1) * HID],
                            mybir.AluOpType.mult, mybir.AluOpType.add,
                        )
                        zt_ps = psum_tr.tile([HID, 128], mdt.float32, tag="tr")
                        nc.tensor.transpose(zt_ps[:], zw[:], ident_t[:])
                        zt = wpool.tile([HID, 128], mdt.float32, tag="zt")
                        nc.scalar.copy(zt[:], zt_ps[:])
                        ps2 = psum_mm.tile([128, HID], mdt.float32, tag="mm")
                        nc.tensor.matmul(
                            ps2[:], zt[:], convT_t[:, l * HID : (l + 1) * HID],
                            start=True, stop=True,
                        )
                        hw = h_sb[:, w * HID : (w + 1) * HID]
                        nc.scalar.activation(hw, ps2[:], mybir.ActivationFunctionType.Relu)
                        if l < L - 1:
                            store_h(w)
                if l < L - 1:
                    allgather()

            # ---- out = h @ w_out.T + b_out ----
            for w in range(NW):
                ht_ps = psum_tr.tile([HID, 128], mdt.float32, tag="tr")
                nc.tensor.transpose(ht_ps[:], h_sb[:, w * HID : (w + 1) * HID], ident_t[:])
                ht = wpool.tile([HID, 128], mdt.float32, tag="zt")
                nc.scalar.copy(ht[:], ht_ps[:])
                ps3 = psum_mm.tile([128, HID], mdt.float32, tag="mm")
                nc.tensor.matmul(ps3[:], ht[:], w_outT_t[:], start=True, stop=True)
                ow = wpool.tile([128, HID], mdt.float32, tag="ow")
                nc.vector.tensor_tensor(ow[:], ps3[:], b_out_t[:], mybir.AluOpType.add)
                nrows = min(NS - w * 128, 128)
                nc.sync.dma_start(out_t[w * 128 : w * 128 + nrows, :], ow[:nrows, :])

    nc.finalize()
    return nc


def kernel(**inputs) -> np.ndarray:
    in_maps, TLs, THs, gidx_lo, gidx_hi, T = _preprocess(
        inputs["x"], inputs["edge_index"], inputs["w_in"], inputs["b_in"],
        inputs["conv_w"], inputs["w_out"], inputs["b_out"],
    )
    nc = _build(TLs, THs, gidx_lo, gidx_hi, T)
    res = run_bass_kernel_spmd(nc, in_maps, list(range(NCORES)))
    out = np.concatenate([res.results[c]["out"] for c in range(NCORES)], axis=0)
    return out



# revision 6
# speedup vs baseline: 1.1725x; 1.1725x over previous
"""GCNII (8 layers, N=50000, E=800000) on 8 trn2 NeuronCores.

Sharding: nodes partitioned into 8 contiguous ranges (6250/core); edges
partitioned by destination so each core owns the scatter-add for its node
range.

v2 design (vs v1 baseline):
- Symmetric norm split: store hs = 0.9*dinv[n] * h[n] (bf16) in h_full; after
  scatter-add multiply by dinv[dst] (per-partition scale on the scalar
  engine). One-hot scatter matrices become BINARY.
- One-hot tiles generated in BULK: one DVE is_equal op per chunk using
  stride-0 broadcast APs (dstw broadcast along columns vs iota broadcast
  along tiles), bf16. Replaces ~940 tiny per-tile vector ops per layer.
- h_full rows are bf16 padded to 128 elems (256B) so dma_gather still works;
  scatter matmuls run in bf16 (f32 PSUM accumulate). z / layer weights stay
  f32 so only ONE bf16 rounding (hs) per layer.
- Final layer computes the output head directly per window (no extra pass,
  no AllGather after the last layer).
"""
import numpy as np
import concourse.bass as bass
import concourse.mybir as mybir
from concourse import bacc, tile
from concourse.bass_utils import run_bass_kernel_spmd

mdt = mybir.dt

N = 50000
E = 800000
FIN = 128
HID = 64
L = 8
ALPHA = 0.1
THETA = 0.5
NCORES = 8
NS = N // NCORES            # 6250 nodes per core
NW = (NS + 127) // 128      # 49 windows per core
NSPAD = NW * 128            # 6272
HALF = 32768                # int16 gather index split
# windows per chunk (gather/oh granularity): 10 chunks of 5,5,...,4
CHUNK_WINDOWS = [5] * 9 + [4]
CHUNKS = len(CHUNK_WINDOWS)
assert sum(CHUNK_WINDOWS) == NW


def _preprocess(x, edge_index, w_in, b_in, conv_w, w_out, b_out):
    row = np.asarray(edge_index[0], dtype=np.int64)
    col = np.asarray(edge_index[1], dtype=np.int64)
    loops = np.arange(N, dtype=np.int64)
    row = np.concatenate([row, loops])
    col = np.concatenate([col, loops])
    deg = np.bincount(col, minlength=N).astype(np.float32)
    dinv = (1.0 / np.sqrt(np.maximum(deg, 1.0))).astype(np.float32)
    dinv = np.where(deg > 0, dinv, 0.0).astype(np.float32)

    per_core = []
    for c in range(NCORES):
        m = (col >= c * NS) & (col < (c + 1) * NS)
        r = row[m]
        d = col[m] - c * NS
        o = np.argsort(d, kind="stable")
        per_core.append((r[o], d[o]))

    # per-window lo/hi counts; tile counts shared across cores
    counts = np.zeros((NCORES, NW, 2), dtype=np.int64)
    bounds = []
    for c in range(NCORES):
        r, d = per_core[c]
        wb = np.searchsorted(d, np.arange(0, NSPAD + 1, 128))
        bounds.append(wb)
        for w in range(NW):
            seg = slice(wb[w], wb[w + 1])
            nlo = int((r[seg] < HALF).sum())
            counts[c, w] = (nlo, (wb[w + 1] - wb[w]) - nlo)
    TLs = np.maximum(np.ceil(counts[:, :, 0].max(axis=0) / 128), 1).astype(np.int64)
    THs = np.maximum(np.ceil(counts[:, :, 1].max(axis=0) / 128), 1).astype(np.int64)

    # global tile order: chunks; within chunk all lo tiles (window-major)
    # then all hi tiles (window-major)
    gidx_lo = np.zeros(NW, dtype=np.int64)
    gidx_hi = np.zeros(NW, dtype=np.int64)
    g = 0
    w0 = 0
    for cw in CHUNK_WINDOWS:
        for w in range(w0, w0 + cw):
            gidx_lo[w] = g
            g += TLs[w]
        for w in range(w0, w0 + cw):
            gidx_hi[w] = g
            g += THs[w]
        w0 += cw
    T = g

    dstw = np.full((NCORES, 128, T), -1.0, dtype=np.float32)
    idx16 = np.zeros((NCORES, 128, 8 * T), dtype=np.int16)
    for c in range(NCORES):
        r, d = per_core[c]
        wb = bounds[c]
        for w in range(NW):
            seg = slice(wb[w], wb[w + 1])
            rs, ds = r[seg], d[seg]
            mlo = rs < HALF
            for p in range(2):
                mask = mlo if p == 0 else ~mlo
                rr = rs[mask] - (0 if p == 0 else HALF)
                dd = ds[mask] - w * 128
                TT = int(TLs[w] if p == 0 else THs[w])
                g0 = int(gidx_lo[w] if p == 0 else gidx_hi[w])
                cap = TT * 128
                rrp = np.zeros(cap, np.int64)
                rrp[: len(rr)] = rr
                ddp = np.full(cap, -1.0, np.float32)
                ddp[: len(dd)] = dd
                for t in range(TT):
                    gg = g0 + t
                    dstw[c, :, gg] = ddp[t * 128 : (t + 1) * 128]
                    v = rrp[t * 128 : (t + 1) * 128].astype(np.int16)
                    idx16[c, :, 8 * gg : 8 * (gg + 1)] = np.tile(
                        v.reshape(8, 16).T, (8, 1)
                    )

    # dense weights (shared across cores)
    w_in = np.asarray(w_in, np.float32)
    conv_w = np.asarray(conv_w, np.float32)
    w_out = np.asarray(w_out, np.float32)
    b_in = np.asarray(b_in, np.float32)
    b_out = np.asarray(b_out, np.float32)
    betas = np.log(THETA / np.arange(1, L + 1, dtype=np.float32) + 1.0)
    convT = np.concatenate(
        [
            ((1.0 - betas[l]) * np.eye(HID, dtype=np.float32) + betas[l] * conv_w[l]).T
            for l in range(L)
        ],
        axis=1,
    )  # [64, L*64]

    def to_bf16(a):
        import ml_dtypes
        return a.astype(ml_dtypes.bfloat16)

    iota = np.tile(np.arange(128, dtype=np.float32), (128, 1))
    consts = {
        "w_inT": np.ascontiguousarray(w_in.T),              # [128, 64]
        "convT": np.ascontiguousarray(convT),               # [64, 512]
        "w_outT": np.ascontiguousarray(w_out.T),            # [64, 64]
        "b_in_rep": np.tile(b_in[None, :], (128, 1)),       # [128, 64]
        "b_out_rep": np.tile(b_out[None, :], (128, 1)),     # [128, 64]
        "iota16": to_bf16(iota),                            # [128, 128] bf16
        "ident": np.eye(128, dtype=np.float32),
    }

    x = np.asarray(x, np.float32)
    in_maps = []
    for c in range(NCORES):
        xT = np.zeros((FIN, NSPAD), np.float32)
        xT[:, :NS] = x[c * NS : (c + 1) * NS].T
        # per-partition 0.9/sqrt(deg) for each window: [128, NW]
        nodes = c * NS + np.arange(NSPAD)
        dv = np.where(np.arange(NSPAD) < NS, dinv[np.minimum(nodes, N - 1)], 0.0)
        dinv09 = np.ascontiguousarray(
            ((1.0 - ALPHA) * dv).reshape(NW, 128).T.astype(np.float32)
        )  # [128, NW]  (0.9*dinv — used when storing hs)
        dinv1 = np.ascontiguousarray(
            dv.reshape(NW, 128).T.astype(np.float32)
        )  # [128, NW]  (plain dinv — dst-side scale in z)
        in_maps.append(
            dict(
                consts,
                xT=np.ascontiguousarray(xT),
                idx16=np.ascontiguousarray(idx16[c]),
                dstw16=np.ascontiguousarray(to_bf16(dstw[c])),
                dinv09=dinv09,
                dinv1=dinv1,
            )
        )
    return in_maps, TLs, THs, gidx_lo, gidx_hi, T


def _ap3(ap2, shape3, strides3):
    """Build a 3-dim AP from a 2-dim slice AP with explicit [stride,size]."""
    return bass.AP(
        ap2.tensor,
        ap2.offset,
        [[strides3[0], shape3[0]], [strides3[1], shape3[1]], [strides3[2], shape3[2]]],
    )


def _build(TLs, THs, gidx_lo, gidx_hi, T, reps=1, sim_single=False):
    nc = bacc.Bacc(None, target_bir_lowering=False, num_devices=NCORES, num_swdge_queues=4)

    xT_in = nc.dram_tensor("xT", [FIN, NSPAD], mdt.float32, kind="ExternalInput")
    idx_in = nc.dram_tensor("idx16", [128, 8 * T], mdt.int16, kind="ExternalInput")
    dstw_in = nc.dram_tensor("dstw16", [128, T], mdt.bfloat16, kind="ExternalInput")
    dinv09_in = nc.dram_tensor("dinv09", [128, NW], mdt.float32, kind="ExternalInput")
    dinv1_in = nc.dram_tensor("dinv1", [128, NW], mdt.float32, kind="ExternalInput")
    w_inT_in = nc.dram_tensor("w_inT", [FIN, HID], mdt.float32, kind="ExternalInput")
    convT_in = nc.dram_tensor("convT", [HID, L * HID], mdt.float32, kind="ExternalInput")
    w_outT_in = nc.dram_tensor("w_outT", [HID, HID], mdt.float32, kind="ExternalInput")
    b_in_in = nc.dram_tensor("b_in_rep", [128, HID], mdt.float32, kind="ExternalInput")
    b_out_in = nc.dram_tensor("b_out_rep", [128, HID], mdt.float32, kind="ExternalInput")
    iota_in = nc.dram_tensor("iota16", [128, 128], mdt.bfloat16, kind="ExternalInput")
    ident_in = nc.dram_tensor("ident", [128, 128], mdt.float32, kind="ExternalInput")

    out_t = nc.dram_tensor("out", [NS, HID], mdt.float32, kind="ExternalOutput")

    bounce = nc.dram_tensor("bounce", [NS, 128], mdt.bfloat16)
    h_full = nc.dram_tensor("h_full", [N, 128], mdt.bfloat16, addr_space="Shared")

    # chunk tile bookkeeping
    chunk_w0 = []
    w0 = 0
    for cw in CHUNK_WINDOWS:
        chunk_w0.append(w0)
        w0 += cw
    NLO = [int(TLs[chunk_w0[s] : chunk_w0[s] + CHUNK_WINDOWS[s]].sum()) for s in range(CHUNKS)]
    NHI = [int(THs[chunk_w0[s] : chunk_w0[s] + CHUNK_WINDOWS[s]].sum()) for s in range(CHUNKS)]
    MAXLO, MAXHI = max(NLO), max(NHI)
    MAXT = max(NLO[s] + NHI[s] for s in range(CHUNKS))

    with tile.TileContext(nc) as tc, \
         tc.tile_pool(name="const", bufs=1) as cpool, \
         tc.tile_pool(name="gath", bufs=2) as gpool, \
         tc.tile_pool(name="oh", bufs=2) as ohpool, \
         tc.tile_pool(name="work", bufs=3) as wpool, \
         tc.tile_pool(name="ps_sc", bufs=3, space="PSUM") as psum_sc, \
         tc.tile_pool(name="ps_tr", bufs=2, space="PSUM") as psum_tr, \
         tc.tile_pool(name="ps_mm", bufs=2, space="PSUM") as psum_mm:

        # ---- persistent constants ----
        iota_t = cpool.tile([128, 128], mdt.bfloat16)
        nc.sync.dma_start(iota_t[:], iota_in[:])
        ident_t = cpool.tile([128, 128], mdt.float32)
        nc.sync.dma_start(ident_t[:], ident_in[:])
        w_inT_t = cpool.tile([FIN, HID], mdt.float32)
        nc.sync.dma_start(w_inT_t[:], w_inT_in[:])
        convT_t = cpool.tile([HID, L * HID], mdt.float32)
        nc.sync.dma_start(convT_t[:], convT_in[:])
        w_outT_t = cpool.tile([HID, HID], mdt.float32)
        nc.sync.dma_start(w_outT_t[:], w_outT_in[:])
        b_in_t = cpool.tile([128, HID], mdt.float32)
        nc.sync.dma_start(b_in_t[:], b_in_in[:])
        b_out_t = cpool.tile([128, HID], mdt.float32)
        nc.sync.dma_start(b_out_t[:], b_out_in[:])
        idx_t = cpool.tile([128, 8 * T], mdt.int16)
        nc.sync.dma_start(idx_t[:], idx_in[:])
        dstw_t = cpool.tile([128, T], mdt.bfloat16)
        nc.sync.dma_start(dstw_t[:], dstw_in[:])
        dinv09_t = cpool.tile([128, NW], mdt.float32)
        nc.sync.dma_start(dinv09_t[:], dinv09_in[:])
        dinv1_t = cpool.tile([128, NW], mdt.float32)
        nc.sync.dma_start(dinv1_t[:], dinv1_in[:])
        xT_t = cpool.tile([FIN, NSPAD], mdt.float32)
        nc.sync.dma_start(xT_t[:], xT_in[:])

        # hs (0.9*dinv*h, bf16, node-major, 128-wide padded rows)
        hs_sb = cpool.tile([128, NW * 128], mdt.bfloat16)
        x0s = cpool.tile([128, NW * HID], mdt.float32)   # 0.1 * h0

        # zero the pad halves of hs_sb once (cols 64:128 of each window)
        nc.vector.memset(hs_sb[:], 0.0)

        def store_hs(w):
            nrows = min(NS - w * 128, 128)
            nc.sync.dma_start(
                bounce[w * 128 : w * 128 + nrows, :],
                hs_sb[:nrows, w * 128 : (w + 1) * 128],
            )

        T_STRIDE = dstw_t[:].ap[0][0]  # partition stride of dstw tile
        I_STRIDE = iota_t[:].ap[0][0]

        for rep_i in range(reps):
            # ---- h0 = relu(x @ w_in.T + b_in); x0s = 0.1*h0; hs = 0.9*dinv*h0 ----
            for w in range(NW):
                ps = psum_mm.tile([128, HID], mdt.float32, tag="mm")
                nc.tensor.matmul(
                    ps[:], xT_t[:, w * 128 : (w + 1) * 128], w_inT_t[:],
                    start=True, stop=True,
                )
                u = wpool.tile([128, HID], mdt.float32, tag="u")
                nc.vector.tensor_tensor(u[:], ps[:], b_in_t[:], mybir.AluOpType.add)
                nc.scalar.activation(
                    x0s[:, w * HID : (w + 1) * HID], u[:],
                    mybir.ActivationFunctionType.Relu, scale=ALPHA,
                )
                nc.scalar.activation(
                    hs_sb[:, w * 128 : w * 128 + HID], u[:],
                    mybir.ActivationFunctionType.Relu,
                    scale=dinv09_t[:, w : w + 1],
                )
                store_hs(w)

            def allgather():
                if sim_single:
                    nc.sync.dma_start(h_full[:NS, :], bounce[:])
                else:
                    nc.gpsimd.collective_compute(
                        "AllGather", mybir.AluOpType.bypass,
                        replica_groups=[list(range(NCORES))],
                        ins=[bounce[:]], outs=[h_full[:]],
                    )

            allgather()

            # ---- layers ----
            qctr = [0]
            for l in range(L):
                last = l == L - 1
                for s in range(CHUNKS):
                    cw = CHUNK_WINDOWS[s]
                    w0 = chunk_w0[s]
                    nlo, nhi = NLO[s], NHI[s]
                    ntc = nlo + nhi
                    base = int(gidx_lo[w0])  # first tile of this chunk
                    glo = gpool.tile([128, MAXLO, 128], mdt.bfloat16, tag="glo")
                    ghi = gpool.tile([128, MAXHI, 128], mdt.bfloat16, tag="ghi")

                    def one_gather(dst, src_ap, tile0, ntiles, nsplit=2):
                        bnds = [tile0 + (ntiles * i) // nsplit for i in range(nsplit + 1)]
                        for i in range(nsplit):
                            a, b = bnds[i], bnds[i + 1]
                            if a == b:
                                continue
                            nidx = (b - a) * 128
                            nc.gpsimd.dma_gather(
                                dst[:, a - tile0 : b - tile0, :], src_ap,
                                idx_t[:, 8 * a : 8 * b], nidx, nidx, 128,
                                single_packet=False, queue_num=qctr[0] % 4,
                            )
                            qctr[0] += 1

                    one_gather(glo, h_full[:, :], base, nlo)
                    one_gather(ghi, h_full[HALF:, :], base + nlo, nhi)

                    # bulk binary one-hot for this chunk's tiles (bf16):
                    # oh[p, t, j] = (dstw[p, base+t] == j)
                    oh = ohpool.tile([128, MAXT * 128], mdt.bfloat16, tag="oh")
                    oh_out = _ap3(oh[:], (128, ntc, 128), (oh[:].ap[0][0], 128, 1))
                    dstw_bc = _ap3(
                        dstw_t[:, base : base + ntc], (128, ntc, 128),
                        (T_STRIDE, 1, 0),
                    )
                    iota_bc = _ap3(
                        iota_t[:], (128, ntc, 128), (I_STRIDE, 0, 1)
                    )
                    nc.vector.tensor_tensor(
                        oh_out, dstw_bc, iota_bc, mybir.AluOpType.is_equal
                    )

                    for wi in range(cw):
                        w = w0 + wi
                        ntiles = int(TLs[w] + THs[w])
                        ps = psum_sc.tile([128, HID], mdt.float32, tag="sc")
                        k = 0
                        for p in range(2):
                            TT = int(TLs[w] if p == 0 else THs[w])
                            g0 = int(gidx_lo[w] if p == 0 else gidx_hi[w])
                            gsrc = glo if p == 0 else ghi
                            for t in range(TT):
                                gg = g0 + t
                                slot = gg - base if p == 0 else gg - base - nlo
                                tloc = gg - base
                                nc.tensor.matmul(
                                    ps[:],
                                    oh[:, tloc * 128 : (tloc + 1) * 128],
                                    gsrc[:, slot, 0:HID],
                                    start=(k == 0), stop=(k == ntiles - 1),
                                )
                                k += 1
                        # zw = dinv09[dst] * ps + x0s ; h = relu(zw @ M_l.T)
                        zw0 = wpool.tile([128, HID], mdt.float32, tag="zw0")
                        nc.scalar.mul(zw0[:], ps[:], dinv1_t[:, w : w + 1])
                        zw = wpool.tile([128, HID], mdt.float32, tag="zw")
                        nc.vector.tensor_tensor(
                            zw[:], zw0[:], x0s[:, w * HID : (w + 1) * HID],
                            mybir.AluOpType.add,
                        )
                        zt_ps = psum_tr.tile([HID, 128], mdt.float32, tag="tr")
                        nc.tensor.transpose(zt_ps[:], zw[:], ident_t[:])
                        zt = wpool.tile([HID, 128], mdt.float32, tag="zt")
                        nc.vector.tensor_copy(zt[:], zt_ps[:])
                        ps2 = psum_mm.tile([128, HID], mdt.float32, tag="mm")
                        nc.tensor.matmul(
                            ps2[:], zt[:], convT_t[:, l * HID : (l + 1) * HID],
                            start=True, stop=True,
                        )
                        if not last:
                            # hs = 0.9*dinv*relu(.) directly into hs_sb (bf16)
                            nc.scalar.activation(
                                hs_sb[:, w * 128 : w * 128 + HID], ps2[:],
                                mybir.ActivationFunctionType.Relu,
                                scale=dinv09_t[:, w : w + 1],
                            )
                            store_hs(w)
                        else:
                            # output head: out = relu(.) @ w_out.T + b_out
                            hf = wpool.tile([128, HID], mdt.float32, tag="hf")
                            nc.scalar.activation(
                                hf[:], ps2[:], mybir.ActivationFunctionType.Relu
                            )
                            ht_ps = psum_tr.tile([HID, 128], mdt.float32, tag="tr")
                            nc.tensor.transpose(ht_ps[:], hf[:], ident_t[:])
                            ht = wpool.tile([HID, 128], mdt.float32, tag="ht")
                            nc.vector.tensor_copy(ht[:], ht_ps[:])
                            ps3 = psum_mm.tile([128, HID], mdt.float32, tag="mm")
                            nc.tensor.matmul(
                                ps3[:], ht[:], w_outT_t[:], start=True, stop=True
                            )
                            ow = wpool.tile([128, HID], mdt.float32, tag="ow")
                            nc.vector.tensor_tensor(
                                ow[:], ps3[:], b_out_t[:], mybir.AluOpType.add
                            )
                            nrows = min(NS - w * 128, 128)
                            nc.sync.dma_start(
                                out_t[w * 128 : w * 128 + nrows, :], ow[:nrows, :]
                            )
                if not last:
                    allgather()

    nc.finalize()
    return nc


def kernel(**inputs) -> np.ndarray:
    in_maps, TLs, THs, gidx_lo, gidx_hi, T = _preprocess(
        inputs["x"], inputs["edge_index"], inputs["w_in"], inputs["b_in"],
        inputs["conv_w"], inputs["w_out"], inputs["b_out"],
    )
    nc = _build(TLs, THs, gidx_lo, gidx_hi, T)
    res = run_bass_kernel_spmd(nc, in_maps, list(range(NCORES)))
    out = np.concatenate([res.results[c]["out"] for c in range(NCORES)], axis=0)
    return out


# revision 7
# speedup vs baseline: 1.4352x; 1.2240x over previous
"""GCNII (8 layers, N=50000, E=800000) on 8 trn2 NeuronCores.

Sharding: nodes partitioned into 8 contiguous ranges (6250/core); edges
partitioned by destination so each core owns the scatter-add for its node
range.

v3 design:
- Symmetric norm split: h_full stores hs = 0.9*dinv[n]*h[n] (bf16); after
  scatter-add multiply by dinv[dst] (per-partition scale on scalar engine).
  One-hot scatter matrices are BINARY.
- One-hot tiles generated in BULK: one DVE is_equal per chunk via stride-0
  broadcast APs, bf16.
- Pair packing: h_full is [N/2, 128] bf16 — row k holds nodes (2k, 2k+1)
  unpadded. Gather idx = src//2 (fits int16, no lo/hi split); edge tiles are
  grouped by src parity so the matmul rhs slices the right half. Collective
  volume halves vs padded rows.
- Self-loops never gathered: their contribution dinv[i]*hs[i] is added
  locally from the resident hs_sb before the dst-side dinv scale.
- z / layer weights stay f32; only ONE bf16 rounding (hs) per layer.
- Final layer computes the output head directly per window.
"""
import numpy as np
import concourse.bass as bass
import concourse.mybir as mybir
from concourse import bacc, tile
from concourse.bass_utils import run_bass_kernel_spmd

mdt = mybir.dt

N = 50000
E = 800000
FIN = 128
HID = 64
L = 8
ALPHA = 0.1
THETA = 0.5
NCORES = 8
NS = N // NCORES            # 6250 nodes per core
NW = (NS + 127) // 128      # 49 windows per core
NSPAD = NW * 128            # 6272
# windows per chunk (gather/oh granularity): 10 chunks of 5,5,...,4
CHUNK_WINDOWS = [5] * 9 + [4]
CHUNKS = len(CHUNK_WINDOWS)
assert sum(CHUNK_WINDOWS) == NW


def _preprocess(x, edge_index, w_in, b_in, conv_w, w_out, b_out):
    row = np.asarray(edge_index[0], dtype=np.int64)
    col = np.asarray(edge_index[1], dtype=np.int64)
    # degree includes the self loops (reference semantics) ...
    loops = np.arange(N, dtype=np.int64)
    deg = np.bincount(np.concatenate([col, loops]), minlength=N).astype(np.float32)
    dinv = (1.0 / np.sqrt(deg)).astype(np.float32)
    # ... but the self loops themselves are handled locally, not gathered.

    per_core = []
    for c in range(NCORES):
        m = (col >= c * NS) & (col < (c + 1) * NS)
        r = row[m]
        d = col[m] - c * NS
        o = np.argsort(d, kind="stable")
        per_core.append((r[o], d[o]))

    # per-window even/odd-parity counts; tile counts shared across cores
    counts = np.zeros((NCORES, NW, 2), dtype=np.int64)
    bounds = []
    for c in range(NCORES):
        r, d = per_core[c]
        wb = np.searchsorted(d, np.arange(0, NSPAD + 1, 128))
        bounds.append(wb)
        for w in range(NW):
            seg = slice(wb[w], wb[w + 1])
            ne = int((r[seg] % 2 == 0).sum())
            counts[c, w] = (ne, (wb[w + 1] - wb[w]) - ne)
    TEs = np.maximum(np.ceil(counts[:, :, 0].max(axis=0) / 128), 1).astype(np.int64)
    TOs = np.maximum(np.ceil(counts[:, :, 1].max(axis=0) / 128), 1).astype(np.int64)

    # global tile order: chunks; within chunk all even tiles (window-major)
    # then all odd tiles (window-major)
    gidx_e = np.zeros(NW, dtype=np.int64)
    gidx_o = np.zeros(NW, dtype=np.int64)
    g = 0
    w0 = 0
    for cw in CHUNK_WINDOWS:
        for w in range(w0, w0 + cw):
            gidx_e[w] = g
            g += TEs[w]
        for w in range(w0, w0 + cw):
            gidx_o[w] = g
            g += TOs[w]
        w0 += cw
    T = g

    dstw = np.full((NCORES, 128, T), -1.0, dtype=np.float32)
    idx16 = np.zeros((NCORES, 128, 8 * T), dtype=np.int16)
    for c in range(NCORES):
        r, d = per_core[c]
        wb = bounds[c]
        for w in range(NW):
            seg = slice(wb[w], wb[w + 1])
            rs, ds = r[seg], d[seg]
            me = rs % 2 == 0
            for p in range(2):
                mask = me if p == 0 else ~me
                rr = rs[mask] // 2
                dd = ds[mask] - w * 128
                TT = int(TEs[w] if p == 0 else TOs[w])
                g0 = int(gidx_e[w] if p == 0 else gidx_o[w])
                cap = TT * 128
                rrp = np.zeros(cap, np.int64)
                rrp[: len(rr)] = rr
                ddp = np.full(cap, -1.0, np.float32)
                ddp[: len(dd)] = dd
                for t in range(TT):
                    gg = g0 + t
                    dstw[c, :, gg] = ddp[t * 128 : (t + 1) * 128]
                    v = rrp[t * 128 : (t + 1) * 128].astype(np.int16)
                    idx16[c, :, 8 * gg : 8 * (gg + 1)] = np.tile(
                        v.reshape(8, 16).T, (8, 1)
                    )

    # dense weights (shared across cores)
    w_in = np.asarray(w_in, np.float32)
    conv_w = np.asarray(conv_w, np.float32)
    w_out = np.asarray(w_out, np.float32)
    b_in = np.asarray(b_in, np.float32)
    b_out = np.asarray(b_out, np.float32)
    betas = np.log(THETA / np.arange(1, L + 1, dtype=np.float32) + 1.0)
    convT = np.concatenate(
        [
            ((1.0 - betas[l]) * np.eye(HID, dtype=np.float32) + betas[l] * conv_w[l]).T
            for l in range(L)
        ],
        axis=1,
    )  # [64, L*64]

    def to_bf16(a):
        import ml_dtypes
        return a.astype(ml_dtypes.bfloat16)

    iota = np.tile(np.arange(128, dtype=np.float32), (128, 1))
    consts = {
        "w_inT": np.ascontiguousarray(w_in.T),              # [128, 64]
        "convT": np.ascontiguousarray(convT),               # [64, 512]
        "w_outT": np.ascontiguousarray(w_out.T),            # [64, 64]
        "b_in_rep": np.tile(b_in[None, :], (128, 1)),       # [128, 64]
        "b_out_rep": np.tile(b_out[None, :], (128, 1)),     # [128, 64]
        "iota16": to_bf16(iota),                            # [128, 128] bf16
        "ident": np.eye(128, dtype=np.float32),
    }

    x = np.asarray(x, np.float32)
    in_maps = []
    for c in range(NCORES):
        xT = np.zeros((FIN, NSPAD), np.float32)
        xT[:, :NS] = x[c * NS : (c + 1) * NS].T
        nodes = c * NS + np.arange(NSPAD)
        dv = np.where(np.arange(NSPAD) < NS, dinv[np.minimum(nodes, N - 1)], 0.0)
        dinv09 = np.ascontiguousarray(
            ((1.0 - ALPHA) * dv).reshape(NW, 128).T.astype(np.float32)
        )  # [128, NW]  (0.9*dinv — used when storing hs)
        dinv1 = np.ascontiguousarray(
            dv.reshape(NW, 128).T.astype(np.float32)
        )  # [128, NW]  (plain dinv — dst-side scale in z)
        in_maps.append(
            dict(
                consts,
                xT=np.ascontiguousarray(xT),
                idx16=np.ascontiguousarray(idx16[c]),
                dstw16=np.ascontiguousarray(to_bf16(dstw[c])),
                dinv09=dinv09,
                dinv1=dinv1,
            )
        )
    return in_maps, TEs, TOs, gidx_e, gidx_o, T


def _ap3(ap2, shape3, strides3):
    """Build a 3-dim AP from a 2-dim slice AP with explicit [stride,size]."""
    return bass.AP(
        ap2.tensor,
        ap2.offset,
        [[strides3[0], shape3[0]], [strides3[1], shape3[1]], [strides3[2], shape3[2]]],
    )


def _build(TEs, TOs, gidx_e, gidx_o, T, reps=1, sim_single=False):
    nc = bacc.Bacc(None, target_bir_lowering=False, num_devices=NCORES, num_swdge_queues=4)

    xT_in = nc.dram_tensor("xT", [FIN, NSPAD], mdt.float32, kind="ExternalInput")
    idx_in = nc.dram_tensor("idx16", [128, 8 * T], mdt.int16, kind="ExternalInput")
    dstw_in = nc.dram_tensor("dstw16", [128, T], mdt.bfloat16, kind="ExternalInput")
    dinv09_in = nc.dram_tensor("dinv09", [128, NW], mdt.float32, kind="ExternalInput")
    dinv1_in = nc.dram_tensor("dinv1", [128, NW], mdt.float32, kind="ExternalInput")
    w_inT_in = nc.dram_tensor("w_inT", [FIN, HID], mdt.float32, kind="ExternalInput")
    convT_in = nc.dram_tensor("convT", [HID, L * HID], mdt.float32, kind="ExternalInput")
    w_outT_in = nc.dram_tensor("w_outT", [HID, HID], mdt.float32, kind="ExternalInput")
    b_in_in = nc.dram_tensor("b_in_rep", [128, HID], mdt.float32, kind="ExternalInput")
    b_out_in = nc.dram_tensor("b_out_rep", [128, HID], mdt.float32, kind="ExternalInput")
    iota_in = nc.dram_tensor("iota16", [128, 128], mdt.bfloat16, kind="ExternalInput")
    ident_in = nc.dram_tensor("ident", [128, 128], mdt.float32, kind="ExternalInput")

    out_t = nc.dram_tensor("out", [NS, HID], mdt.float32, kind="ExternalOutput")

    # pair-packed hs: row k of h_full holds nodes (2k, 2k+1), 64 bf16 each
    bounce = nc.dram_tensor("bounce", [NS // 2, 2 * HID], mdt.bfloat16)
    h_full = nc.dram_tensor("h_full", [N // 2, 2 * HID], mdt.bfloat16, addr_space="Shared")

    # chunk tile bookkeeping
    chunk_w0 = []
    w0 = 0
    for cw in CHUNK_WINDOWS:
        chunk_w0.append(w0)
        w0 += cw
    NE = [int(TEs[chunk_w0[s] : chunk_w0[s] + CHUNK_WINDOWS[s]].sum()) for s in range(CHUNKS)]
    NO = [int(TOs[chunk_w0[s] : chunk_w0[s] + CHUNK_WINDOWS[s]].sum()) for s in range(CHUNKS)]
    MAXT = max(NE[s] + NO[s] for s in range(CHUNKS))

    with tile.TileContext(nc) as tc, \
         tc.tile_pool(name="const", bufs=1) as cpool, \
         tc.tile_pool(name="gath", bufs=2) as gpool, \
         tc.tile_pool(name="oh", bufs=2) as ohpool, \
         tc.tile_pool(name="work", bufs=3) as wpool, \
         tc.tile_pool(name="ps_sc", bufs=3, space="PSUM") as psum_sc, \
         tc.tile_pool(name="ps_tr", bufs=2, space="PSUM") as psum_tr, \
         tc.tile_pool(name="ps_mm", bufs=2, space="PSUM") as psum_mm:

        # ---- persistent constants ----
        iota_t = cpool.tile([128, 128], mdt.bfloat16)
        nc.sync.dma_start(iota_t[:], iota_in[:])
        ident_t = cpool.tile([128, 128], mdt.float32)
        nc.sync.dma_start(ident_t[:], ident_in[:])
        w_inT_t = cpool.tile([FIN, HID], mdt.float32)
        nc.sync.dma_start(w_inT_t[:], w_inT_in[:])
        convT_t = cpool.tile([HID, L * HID], mdt.float32)
        nc.sync.dma_start(convT_t[:], convT_in[:])
        w_outT_t = cpool.tile([HID, HID], mdt.float32)
        nc.sync.dma_start(w_outT_t[:], w_outT_in[:])
        b_in_t = cpool.tile([128, HID], mdt.float32)
        nc.sync.dma_start(b_in_t[:], b_in_in[:])
        b_out_t = cpool.tile([128, HID], mdt.float32)
        nc.sync.dma_start(b_out_t[:], b_out_in[:])
        idx_t = cpool.tile([128, 8 * T], mdt.int16)
        nc.sync.dma_start(idx_t[:], idx_in[:])
        dstw_t = cpool.tile([128, T], mdt.bfloat16)
        nc.sync.dma_start(dstw_t[:], dstw_in[:])
        dinv09_t = cpool.tile([128, NW], mdt.float32)
        nc.sync.dma_start(dinv09_t[:], dinv09_in[:])
        dinv1_t = cpool.tile([128, NW], mdt.float32)
        nc.sync.dma_start(dinv1_t[:], dinv1_in[:])
        xT_t = cpool.tile([FIN, NSPAD], mdt.float32)
        nc.sync.dma_start(xT_t[:], xT_in[:])

        # hs (0.9*dinv*h, bf16, node-major, unpadded 64-wide windows)
        hs_sb = cpool.tile([128, NW * HID], mdt.bfloat16)
        x0s = cpool.tile([128, NW * HID], mdt.float32)   # 0.1 * h0

        # zero once: pad rows of the last window read as 0 by the self-term
        nc.vector.memset(hs_sb[:], 0.0)

        def store_hs(w):
            nrows = min(NS - w * 128, 128)
            nc.sync.dma_start(
                bounce[w * 64 : w * 64 + nrows // 2, :],
                hs_sb[:nrows, w * HID : (w + 1) * HID],
            )

        T_STRIDE = dstw_t[:].ap[0][0]  # partition stride of dstw tile
        I_STRIDE = iota_t[:].ap[0][0]

        for rep_i in range(reps):
            # ---- h0 = relu(x @ w_in.T + b_in); x0s = 0.1*h0; hs = 0.9*dinv*h0 ----
            for w in range(NW):
                ps = psum_mm.tile([128, HID], mdt.float32, tag="mm")
                nc.tensor.matmul(
                    ps[:], xT_t[:, w * 128 : (w + 1) * 128], w_inT_t[:],
                    start=True, stop=True,
                )
                u = wpool.tile([128, HID], mdt.float32, tag="u")
                nc.vector.tensor_tensor(u[:], ps[:], b_in_t[:], mybir.AluOpType.add)
                nc.scalar.activation(
                    x0s[:, w * HID : (w + 1) * HID], u[:],
                    mybir.ActivationFunctionType.Relu, scale=ALPHA,
                )
                nc.scalar.activation(
                    hs_sb[:, w * HID : (w + 1) * HID], u[:],
                    mybir.ActivationFunctionType.Relu,
                    scale=dinv09_t[:, w : w + 1],
                )
                store_hs(w)

            def allgather():
                if sim_single:
                    nc.sync.dma_start(h_full[: NS // 2, :], bounce[:])
                else:
                    nc.gpsimd.collective_compute(
                        "AllGather", mybir.AluOpType.bypass,
                        replica_groups=[list(range(NCORES))],
                        ins=[bounce[:]], outs=[h_full[:]],
                    )

            allgather()

            # ---- layers ----
            qctr = [0]
            for l in range(L):
                last = l == L - 1
                for s in range(CHUNKS):
                    cw = CHUNK_WINDOWS[s]
                    w0 = chunk_w0[s]
                    ne, no = NE[s], NO[s]
                    ntc = ne + no
                    base = int(gidx_e[w0])  # first tile of this chunk
                    gbuf = gpool.tile([128, MAXT, 128], mdt.bfloat16, tag="g")

                    nsplit = 4
                    bnds = [base + (ntc * i) // nsplit for i in range(nsplit + 1)]
                    for i in range(nsplit):
                        a, b = bnds[i], bnds[i + 1]
                        if a == b:
                            continue
                        nidx = (b - a) * 128
                        nc.gpsimd.dma_gather(
                            gbuf[:, a - base : b - base, :], h_full[:, :],
                            idx_t[:, 8 * a : 8 * b], nidx, nidx, 128,
                            single_packet=False, queue_num=qctr[0] % 4,
                        )
                        qctr[0] += 1

                    # bulk binary one-hot for this chunk's tiles (bf16):
                    # oh[p, t, j] = (dstw[p, base+t] == j)
                    oh = ohpool.tile([128, MAXT * 128], mdt.bfloat16, tag="oh")
                    oh_out = _ap3(oh[:], (128, ntc, 128), (oh[:].ap[0][0], 128, 1))
                    dstw_bc = _ap3(
                        dstw_t[:, base : base + ntc], (128, ntc, 128),
                        (T_STRIDE, 1, 0),
                    )
                    iota_bc = _ap3(
                        iota_t[:], (128, ntc, 128), (I_STRIDE, 0, 1)
                    )
                    nc.vector.tensor_tensor(
                        oh_out, dstw_bc, iota_bc, mybir.AluOpType.is_equal
                    )

                    for wi in range(cw):
                        w = w0 + wi
                        ntiles = int(TEs[w] + TOs[w])
                        ps = psum_sc.tile([128, HID], mdt.float32, tag="sc")
                        k = 0
                        for p in range(2):
                            TT = int(TEs[w] if p == 0 else TOs[w])
                            g0 = int(gidx_e[w] if p == 0 else gidx_o[w])
                            poff = 0 if p == 0 else HID
                            for t in range(TT):
                                gg = g0 + t
                                tloc = gg - base
                                nc.tensor.matmul(
                                    ps[:],
                                    oh[:, tloc * 128 : (tloc + 1) * 128],
                                    gbuf[:, tloc, poff : poff + HID],
                                    start=(k == 0), stop=(k == ntiles - 1),
                                )
                                k += 1
                        # z = dinv1[dst]*(ps + hs_self) + x0s ; h = relu(z @ M_l.T)
                        s1 = wpool.tile([128, HID], mdt.float32, tag="s1")
                        nc.vector.tensor_tensor(
                            s1[:], ps[:], hs_sb[:, w * HID : (w + 1) * HID],
                            mybir.AluOpType.add,
                        )
                        zw0 = wpool.tile([128, HID], mdt.float32, tag="zw0")
                        nc.scalar.mul(zw0[:], s1[:], dinv1_t[:, w : w + 1])
                        zw = wpool.tile([128, HID], mdt.float32, tag="zw")
                        nc.vector.tensor_tensor(
                            zw[:], zw0[:], x0s[:, w * HID : (w + 1) * HID],
                            mybir.AluOpType.add,
                        )
                        zt_ps = psum_tr.tile([HID, 128], mdt.float32, tag="tr")
                        nc.tensor.transpose(zt_ps[:], zw[:], ident_t[:])
                        zt = wpool.tile([HID, 128], mdt.float32, tag="zt")
                        nc.vector.tensor_copy(zt[:], zt_ps[:])
                        ps2 = psum_mm.tile([128, HID], mdt.float32, tag="mm")
                        nc.tensor.matmul(
                            ps2[:], zt[:], convT_t[:, l * HID : (l + 1) * HID],
                            start=True, stop=True,
                        )
                        if not last:
                            # hs = 0.9*dinv*relu(.) directly into hs_sb (bf16)
                            nc.scalar.activation(
                                hs_sb[:, w * HID : (w + 1) * HID], ps2[:],
                                mybir.ActivationFunctionType.Relu,
                                scale=dinv09_t[:, w : w + 1],
                            )
                            store_hs(w)
                        else:
                            # output head: out = relu(.) @ w_out.T + b_out
                            hf = wpool.tile([128, HID], mdt.float32, tag="hf")
                            nc.scalar.activation(
                                hf[:], ps2[:], mybir.ActivationFunctionType.Relu
                            )
                            ht_ps = psum_tr.tile([HID, 128], mdt.float32, tag="tr")
                            nc.tensor.transpose(ht_ps[:], hf[:], ident_t[:])
                            ht = wpool.tile([HID, 128], mdt.float32, tag="ht")
                            nc.vector.tensor_copy(ht[:], ht_ps[:])
                            ps3 = psum_mm.tile([128, HID], mdt.float32, tag="mm")
                            nc.tensor.matmul(
                                ps3[:], ht[:], w_outT_t[:], start=True, stop=True
                            )
                            ow = wpool.tile([128, HID], mdt.float32, tag="ow")
                            nc.vector.tensor_tensor(
                                ow[:], ps3[:], b_out_t[:], mybir.AluOpType.add
                            )
                            nrows = min(NS - w * 128, 128)
                            nc.sync.dma_start(
                                out_t[w * 128 : w * 128 + nrows, :], ow[:nrows, :]
                            )
                if not last:
                    allgather()

    nc.finalize()
    return nc


def kernel(**inputs) -> np.ndarray:
    in_maps, TEs, TOs, gidx_e, gidx_o, T = _preprocess(
        inputs["x"], inputs["edge_index"], inputs["w_in"], inputs["b_in"],
        inputs["conv_w"], inputs["w_out"], inputs["b_out"],
    )
    nc = _build(TEs, TOs, gidx_e, gidx_o, T)
    res = run_bass_kernel_spmd(nc, in_maps, list(range(NCORES)))
    out = np.concatenate([res.results[c]["out"] for c in range(NCORES)], axis=0)
    return out


# revision 10
# speedup vs baseline: 2.0404x; 1.4217x over previous
"""GCNII (8 layers, N=50000, E=800000) on 8 trn2 NeuronCores.

Sharding: nodes partitioned into 8 contiguous ranges (6250/core); edges
partitioned by destination so each core owns the scatter-add for its node
range.

v3 design:
- Symmetric norm split: h_full stores hs = 0.9*dinv[n]*h[n] (bf16); after
  scatter-add multiply by dinv[dst] (per-partition scale on scalar engine).
  One-hot scatter matrices are BINARY.
- One-hot tiles generated in BULK: one DVE is_equal per chunk via stride-0
  broadcast APs, bf16.
- Pair packing: h_full is [N/2, 128] bf16 — row k holds nodes (2k, 2k+1)
  unpadded. Gather idx = src//2 (fits int16, no lo/hi split); edge tiles are
  grouped by src parity so the matmul rhs slices the right half. Collective
  volume halves vs padded rows.
- Self-loops never gathered: their contribution dinv[i]*hs[i] is added
  locally from the resident hs_sb before the dst-side dinv scale.
- z / layer weights stay f32; only ONE bf16 rounding (hs) per layer.
- Final layer computes the output head directly per window.
"""
import numpy as np
import concourse.bass as bass
import concourse.mybir as mybir
from concourse import bacc, tile
from concourse.bass_utils import run_bass_kernel_spmd

mdt = mybir.dt

N = 50000
E = 800000
FIN = 128
HID = 64
L = 8
ALPHA = 0.1
THETA = 0.5
NCORES = 8
NS = N // NCORES            # 6250 nodes per core
NW = (NS + 127) // 128      # 49 windows per core
NSPAD = NW * 128            # 6272
# windows per chunk (gather/oh granularity): 10 chunks of 5,5,...,4
CHUNK_WINDOWS = [5] * 9 + [4]
CHUNKS = len(CHUNK_WINDOWS)
assert sum(CHUNK_WINDOWS) == NW


def _preprocess(x, edge_index, w_in, b_in, conv_w, w_out, b_out):
    row = np.asarray(edge_index[0], dtype=np.int64)
    col = np.asarray(edge_index[1], dtype=np.int64)
    # degree includes the self loops (reference semantics) ...
    loops = np.arange(N, dtype=np.int64)
    deg = np.bincount(np.concatenate([col, loops]), minlength=N).astype(np.float32)
    dinv = (1.0 / np.sqrt(deg)).astype(np.float32)
    # ... but the self loops themselves are handled locally, not gathered.

    per_core = []
    for c in range(NCORES):
        m = (col >= c * NS) & (col < (c + 1) * NS)
        r = row[m]
        d = col[m] - c * NS
        o = np.argsort(d, kind="stable")
        per_core.append((r[o], d[o]))

    # per-window even/odd-parity counts; tile counts shared across cores
    counts = np.zeros((NCORES, NW, 2), dtype=np.int64)
    bounds = []
    for c in range(NCORES):
        r, d = per_core[c]
        wb = np.searchsorted(d, np.arange(0, NSPAD + 1, 128))
        bounds.append(wb)
        for w in range(NW):
            seg = slice(wb[w], wb[w + 1])
            ne = int((r[seg] % 2 == 0).sum())
            counts[c, w] = (ne, (wb[w + 1] - wb[w]) - ne)
    TEs = np.maximum(np.ceil(counts[:, :, 0].max(axis=0) / 128), 1).astype(np.int64)
    TOs = np.maximum(np.ceil(counts[:, :, 1].max(axis=0) / 128), 1).astype(np.int64)

    # global tile order: chunks; within chunk all even tiles (window-major)
    # then all odd tiles (window-major)
    gidx_e = np.zeros(NW, dtype=np.int64)
    gidx_o = np.zeros(NW, dtype=np.int64)
    g = 0
    w0 = 0
    for cw in CHUNK_WINDOWS:
        for w in range(w0, w0 + cw):
            gidx_e[w] = g
            g += TEs[w]
        for w in range(w0, w0 + cw):
            gidx_o[w] = g
            g += TOs[w]
        w0 += cw
    T = g

    dstw = np.full((NCORES, 128, T), -1.0, dtype=np.float32)
    idx16 = np.zeros((NCORES, 128, 8 * T), dtype=np.int16)
    gcnt = np.zeros((NCORES, 1, NW * 2), dtype=np.int32)
    for c in range(NCORES):
        r, d = per_core[c]
        wb = bounds[c]
        for w in range(NW):
            seg = slice(wb[w], wb[w + 1])
            rs, ds = r[seg], d[seg]
            me = rs % 2 == 0
            for p in range(2):
                mask = me if p == 0 else ~me
                rr = rs[mask] // 2
                dd = ds[mask] - w * 128
                TT = int(TEs[w] if p == 0 else TOs[w])
                g0 = int(gidx_e[w] if p == 0 else gidx_o[w])
                gcnt[c, 0, w * 2 + p] = max(len(rr), 1)
                cap = TT * 128
                rrp = np.full(cap, -1, np.int64)
                rrp[: len(rr)] = rr
                ddp = np.full(cap, -1.0, np.float32)
                ddp[: len(dd)] = dd
                for t in range(TT):
                    gg = g0 + t
                    dstw[c, :, gg] = ddp[t * 128 : (t + 1) * 128]
                    v = rrp[t * 128 : (t + 1) * 128].astype(np.int16)
                    idx16[c, :, 8 * gg : 8 * (gg + 1)] = np.tile(
                        v.reshape(8, 16).T, (8, 1)
                    )

    # dense weights (shared across cores)
    w_in = np.asarray(w_in, np.float32)
    conv_w = np.asarray(conv_w, np.float32)
    w_out = np.asarray(w_out, np.float32)
    b_in = np.asarray(b_in, np.float32)
    b_out = np.asarray(b_out, np.float32)
    betas = np.log(THETA / np.arange(1, L + 1, dtype=np.float32) + 1.0)
    convT = np.concatenate(
        [
            ((1.0 - betas[l]) * np.eye(HID, dtype=np.float32) + betas[l] * conv_w[l]).T
            for l in range(L)
        ],
        axis=1,
    )  # [64, L*64]

    def to_bf16(a):
        import ml_dtypes
        return a.astype(ml_dtypes.bfloat16)

    iota = np.tile(np.arange(128, dtype=np.float32), (128, 1))
    consts = {
        "w_inT": np.ascontiguousarray(w_in.T),              # [128, 64]
        "convT": np.ascontiguousarray(convT),               # [64, 512]
        "w_outT": np.ascontiguousarray(w_out.T),            # [64, 64]
        "b_in_rep": np.tile(b_in[None, :], (128, 1)),       # [128, 64]
        "b_out_rep": np.tile(b_out[None, :], (128, 1)),     # [128, 64]
        "iota16": to_bf16(iota),                            # [128, 128] bf16
        "ident": np.eye(128, dtype=np.float32),
    }

    x = np.asarray(x, np.float32)
    in_maps = []
    for c in range(NCORES):
        xT = np.zeros((FIN, NSPAD), np.float32)
        xT[:, :NS] = x[c * NS : (c + 1) * NS].T
        nodes = c * NS + np.arange(NSPAD)
        dv = np.where(np.arange(NSPAD) < NS, dinv[np.minimum(nodes, N - 1)], 0.0)
        dinv09 = np.ascontiguousarray(
            ((1.0 - ALPHA) * dv).reshape(NW, 128).T.astype(np.float32)
        )  # [128, NW]  (0.9*dinv — used when storing hs)
        dinv1 = np.ascontiguousarray(
            dv.reshape(NW, 128).T.astype(np.float32)
        )  # [128, NW]  (plain dinv — dst-side scale in z)
        in_maps.append(
            dict(
                consts,
                xT=np.ascontiguousarray(xT),
                idx16=np.ascontiguousarray(idx16[c]),
                gcnt=np.ascontiguousarray(gcnt[c]),
                dstw16=np.ascontiguousarray(to_bf16(dstw[c])),
                dinv09=dinv09,
                dinv1=dinv1,
            )
        )
    return in_maps, TEs, TOs, gidx_e, gidx_o, T


def _ap3(ap2, shape3, strides3):
    """Build a 3-dim AP from a 2-dim slice AP with explicit [stride,size]."""
    return bass.AP(
        ap2.tensor,
        ap2.offset,
        [[strides3[0], shape3[0]], [strides3[1], shape3[1]], [strides3[2], shape3[2]]],
    )


def _build(TEs, TOs, gidx_e, gidx_o, T, reps=1, sim_single=False):
    nc = bacc.Bacc(None, target_bir_lowering=False, num_devices=NCORES, num_swdge_queues=4)

    xT_in = nc.dram_tensor("xT", [FIN, NSPAD], mdt.float32, kind="ExternalInput")
    idx_in = nc.dram_tensor("idx16", [128, 8 * T], mdt.int16, kind="ExternalInput")
    dstw_in = nc.dram_tensor("dstw16", [128, T], mdt.bfloat16, kind="ExternalInput")
    dinv09_in = nc.dram_tensor("dinv09", [128, NW], mdt.float32, kind="ExternalInput")
    dinv1_in = nc.dram_tensor("dinv1", [128, NW], mdt.float32, kind="ExternalInput")
    gcnt_in = nc.dram_tensor("gcnt", [1, NW * 2], mdt.int32, kind="ExternalInput")
    w_inT_in = nc.dram_tensor("w_inT", [FIN, HID], mdt.float32, kind="ExternalInput")
    convT_in = nc.dram_tensor("convT", [HID, L * HID], mdt.float32, kind="ExternalInput")
    w_outT_in = nc.dram_tensor("w_outT", [HID, HID], mdt.float32, kind="ExternalInput")
    b_in_in = nc.dram_tensor("b_in_rep", [128, HID], mdt.float32, kind="ExternalInput")
    b_out_in = nc.dram_tensor("b_out_rep", [128, HID], mdt.float32, kind="ExternalInput")
    iota_in = nc.dram_tensor("iota16", [128, 128], mdt.bfloat16, kind="ExternalInput")
    ident_in = nc.dram_tensor("ident", [128, 128], mdt.float32, kind="ExternalInput")

    out_t = nc.dram_tensor("out", [NS, HID], mdt.float32, kind="ExternalOutput")

    # pair-packed hs: row k of h_full holds nodes (2k, 2k+1), 64 bf16 each
    bounce = nc.dram_tensor("bounce", [NS // 2, 2 * HID], mdt.bfloat16)
    h_full = nc.dram_tensor("h_full", [N // 2, 2 * HID], mdt.bfloat16, addr_space="Shared")

    # chunk tile bookkeeping
    chunk_w0 = []
    w0 = 0
    for cw in CHUNK_WINDOWS:
        chunk_w0.append(w0)
        w0 += cw
    NE = [int(TEs[chunk_w0[s] : chunk_w0[s] + CHUNK_WINDOWS[s]].sum()) for s in range(CHUNKS)]
    NO = [int(TOs[chunk_w0[s] : chunk_w0[s] + CHUNK_WINDOWS[s]].sum()) for s in range(CHUNKS)]
    MAXT = max(NE[s] + NO[s] for s in range(CHUNKS))

    with tile.TileContext(nc) as tc, \
         tc.tile_pool(name="const", bufs=1) as cpool, \
         tc.tile_pool(name="gath", bufs=2) as gpool, \
         tc.tile_pool(name="oh", bufs=2) as ohpool, \
         tc.tile_pool(name="work", bufs=3) as wpool, \
         tc.tile_pool(name="ps_sc", bufs=3, space="PSUM") as psum_sc, \
         tc.tile_pool(name="ps_tr", bufs=2, space="PSUM") as psum_tr, \
         tc.tile_pool(name="ps_mm", bufs=2, space="PSUM") as psum_mm:

        # ---- persistent constants ----
        iota_t = cpool.tile([128, 128], mdt.bfloat16)
        nc.sync.dma_start(iota_t[:], iota_in[:])
        ident_t = cpool.tile([128, 128], mdt.float32)
        nc.sync.dma_start(ident_t[:], ident_in[:])
        w_inT_t = cpool.tile([FIN, HID], mdt.float32)
        nc.sync.dma_start(w_inT_t[:], w_inT_in[:])
        convT_t = cpool.tile([HID, L * HID], mdt.float32)
        nc.sync.dma_start(convT_t[:], convT_in[:])
        w_outT_t = cpool.tile([HID, HID], mdt.float32)
        nc.sync.dma_start(w_outT_t[:], w_outT_in[:])
        b_in_t = cpool.tile([128, HID], mdt.float32)
        nc.sync.dma_start(b_in_t[:], b_in_in[:])
        b_out_t = cpool.tile([128, HID], mdt.float32)
        nc.sync.dma_start(b_out_t[:], b_out_in[:])
        idx_t = cpool.tile([128, 8 * T], mdt.int16)
        nc.sync.dma_start(idx_t[:], idx_in[:])
        dstw_t = cpool.tile([128, T], mdt.bfloat16)
        nc.sync.dma_start(dstw_t[:], dstw_in[:])
        dinv09_t = cpool.tile([128, NW], mdt.float32)
        nc.sync.dma_start(dinv09_t[:], dinv09_in[:])
        dinv1_t = cpool.tile([128, NW], mdt.float32)
        nc.sync.dma_start(dinv1_t[:], dinv1_in[:])
        gcnt_t = cpool.tile([1, NW * 2], mdt.int32)
        nc.sync.dma_start(gcnt_t[:], gcnt_in[:])
        xT_t = cpool.tile([FIN, NSPAD], mdt.float32)
        nc.sync.dma_start(xT_t[:], xT_in[:])

        # hs (0.9*dinv*h, bf16, node-major, unpadded 64-wide windows)
        hs_sb = cpool.tile([128, NW * HID], mdt.bfloat16)
        x0s = cpool.tile([128, NW * HID], mdt.float32)   # 0.1 * h0

        # zero once: pad rows of the last window read as 0 by the self-term
        nc.vector.memset(hs_sb[:], 0.0)

        def store_hs(w):
            nrows = min(NS - w * 128, 128)
            nc.sync.dma_start(
                bounce[w * 64 : w * 64 + nrows // 2, :],
                hs_sb[:nrows, w * HID : (w + 1) * HID],
            )

        T_STRIDE = dstw_t[:].ap[0][0]  # partition stride of dstw tile
        I_STRIDE = iota_t[:].ap[0][0]

        # round-robin count registers for the gathers (Pool runs in order)
        cnt_regs = [nc.gpsimd.alloc_register(f"gcnt_r{i}") for i in range(2)]

        for rep_i in range(reps):
            # ---- h0 = relu(x @ w_in.T + b_in); x0s = 0.1*h0; hs = 0.9*dinv*h0 ----
            for w in range(NW):
                ps = psum_mm.tile([128, HID], mdt.float32, tag="mm")
                nc.tensor.matmul(
                    ps[:], xT_t[:, w * 128 : (w + 1) * 128], w_inT_t[:],
                    start=True, stop=True,
                )
                u = wpool.tile([128, HID], mdt.float32, tag="u")
                nc.vector.tensor_tensor(u[:], ps[:], b_in_t[:], mybir.AluOpType.add)
                nc.scalar.activation(
                    x0s[:, w * HID : (w + 1) * HID], u[:],
                    mybir.ActivationFunctionType.Relu, scale=ALPHA,
                )
                nc.scalar.activation(
                    hs_sb[:, w * HID : (w + 1) * HID], u[:],
                    mybir.ActivationFunctionType.Relu,
                    scale=dinv09_t[:, w : w + 1],
                )
                store_hs(w)

            def allgather():
                if sim_single:
                    nc.sync.dma_start(h_full[: NS // 2, :], bounce[:])
                else:
                    nc.gpsimd.collective_compute(
                        "AllGather", mybir.AluOpType.bypass,
                        replica_groups=[list(range(NCORES))],
                        ins=[bounce[:]], outs=[h_full[:]],
                    )

            allgather()

            # ---- layers ----
            qctr = [0]
            for l in range(L):
                last = l == L - 1
                for s in range(CHUNKS):
                    cw = CHUNK_WINDOWS[s]
                    w0 = chunk_w0[s]
                    ne, no = NE[s], NO[s]
                    ntc = ne + no
                    base = int(gidx_e[w0])  # first tile of this chunk
                    gbuf = gpool.tile([128, MAXT, 128], mdt.bfloat16, tag="g")
                    if l == 0 and s < 2:
                        nc.vector.memset(gbuf[:], 0.0)

                    for wi in range(cw):
                        w = w0 + wi
                        for p in range(2):
                            TT = int(TEs[w] if p == 0 else TOs[w])
                            g0 = int(gidx_e[w] if p == 0 else gidx_o[w])
                            cr = cnt_regs[qctr[0] % 2]
                            nc.gpsimd.reg_load(
                                cr, gcnt_t[0:1, w * 2 + p : w * 2 + p + 1]
                            )
                            nc.gpsimd.dma_gather(
                                gbuf[:, g0 - base : g0 - base + TT, :], h_full[:, :],
                                idx_t[:, 8 * g0 : 8 * (g0 + TT)], TT * 128, cr, 128,
                                single_packet=False, queue_num=qctr[0] % 4,
                            )
                            qctr[0] += 1

                    # bulk binary one-hot for this chunk's tiles (bf16):
                    # oh[p, t, j] = (dstw[p, base+t] == j)
                    oh = ohpool.tile([128, MAXT * 128], mdt.bfloat16, tag="oh")
                    oh_out = _ap3(oh[:], (128, ntc, 128), (oh[:].ap[0][0], 128, 1))
                    dstw_bc = _ap3(
                        dstw_t[:, base : base + ntc], (128, ntc, 128),
                        (T_STRIDE, 1, 0),
                    )
                    iota_bc = _ap3(
                        iota_t[:], (128, ntc, 128), (I_STRIDE, 0, 1)
                    )
                    nc.vector.tensor_tensor(
                        oh_out, dstw_bc, iota_bc, mybir.AluOpType.is_equal
                    )

                    for wi in range(cw):
                        w = w0 + wi
                        ntiles = int(TEs[w] + TOs[w])
                        ps = psum_sc.tile([128, HID], mdt.float32, tag="sc")
                        k = 0
                        for p in range(2):
                            TT = int(TEs[w] if p == 0 else TOs[w])
                            g0 = int(gidx_e[w] if p == 0 else gidx_o[w])
                            poff = 0 if p == 0 else HID
                            for t in range(TT):
                                gg = g0 + t
                                tloc = gg - base
                                nc.tensor.matmul(
                                    ps[:],
                                    oh[:, tloc * 128 : (tloc + 1) * 128],
                                    gbuf[:, tloc, poff : poff + HID],
                                    start=(k == 0), stop=(k == ntiles - 1),
                                )
                                k += 1
                        # z = dinv1[dst]*(ps + hs_self) + x0s ; h = relu(z @ M_l.T)
                        s1 = wpool.tile([128, HID], mdt.float32, tag="s1")
                        nc.vector.tensor_tensor(
                            s1[:], ps[:], hs_sb[:, w * HID : (w + 1) * HID],
                            mybir.AluOpType.add,
                        )
                        zw0 = wpool.tile([128, HID], mdt.float32, tag="zw0")
                        nc.scalar.mul(zw0[:], s1[:], dinv1_t[:, w : w + 1])
                        zw = wpool.tile([128, HID], mdt.float32, tag="zw")
                        nc.vector.tensor_tensor(
                            zw[:], zw0[:], x0s[:, w * HID : (w + 1) * HID],
                            mybir.AluOpType.add,
                        )
                        zt_ps = psum_tr.tile([HID, 128], mdt.float32, tag="tr")
                        nc.tensor.transpose(zt_ps[:], zw[:], ident_t[:])
                        zt = wpool.tile([HID, 128], mdt.float32, tag="zt")
                        nc.vector.tensor_copy(zt[:], zt_ps[:])
                        ps2 = psum_mm.tile([128, HID], mdt.float32, tag="mm")
                        nc.tensor.matmul(
                            ps2[:], zt[:], convT_t[:, l * HID : (l + 1) * HID],
                            start=True, stop=True,
                        )
                        if not last:
                            # hs = 0.9*dinv*relu(.) directly into hs_sb (bf16)
                            nc.scalar.activation(
                                hs_sb[:, w * HID : (w + 1) * HID], ps2[:],
                                mybir.ActivationFunctionType.Relu,
                                scale=dinv09_t[:, w : w + 1],
                            )
                            store_hs(w)
                        else:
                            # output head: out = relu(.) @ w_out.T + b_out
                            hf = wpool.tile([128, HID], mdt.float32, tag="hf")
                            nc.scalar.activation(
                                hf[:], ps2[:], mybir.ActivationFunctionType.Relu
                            )
                            ht_ps = psum_tr.tile([HID, 128], mdt.float32, tag="tr")
                            nc.tensor.transpose(ht_ps[:], hf[:], ident_t[:])
                            ht = wpool.tile([HID, 128], mdt.float32, tag="ht")
                            nc.vector.tensor_copy(ht[:], ht_ps[:])
                            ps3 = psum_mm.tile([128, HID], mdt.float32, tag="mm")
                            nc.tensor.matmul(
                                ps3[:], ht[:], w_outT_t[:], start=True, stop=True
                            )
                            ow = wpool.tile([128, HID], mdt.float32, tag="ow")
                            nc.vector.tensor_tensor(
                                ow[:], ps3[:], b_out_t[:], mybir.AluOpType.add
                            )
                            nrows = min(NS - w * 128, 128)
                            nc.sync.dma_start(
                                out_t[w * 128 : w * 128 + nrows, :], ow[:nrows, :]
                            )
                if not last:
                    allgather()

    nc.finalize()
    return nc


def kernel(**inputs) -> np.ndarray:
    in_maps, TEs, TOs, gidx_e, gidx_o, T = _preprocess(
        inputs["x"], inputs["edge_index"], inputs["w_in"], inputs["b_in"],
        inputs["conv_w"], inputs["w_out"], inputs["b_out"],
    )
    nc = _build(TEs, TOs, gidx_e, gidx_o, T)
    res = run_bass_kernel_spmd(nc, in_maps, list(range(NCORES)))
    out = np.concatenate([res.results[c]["out"] for c in range(NCORES)], axis=0)
    return out
